# revision 61
# baseline (speedup 1.0000x reference)
"""Trainium2 Bass kernel for a Neural CDE (fixed-step RK4 over a cubic spline).

Strategy (v3)
-------------
Pure data-parallel over batch: 4096 samples -> 8 NeuronCores x 512.
Per core, activations live feature-major in SBUF: [C=128 partitions, B free].
The 512-sample slice is split into NSUB chains (default 4 x 128) pipelined
against each other: each RK4 step is a serial engine chain, so wall clock
~ n_steps * chain_latency; the chains fill the engines inside that latency.

Everything is fp16 (validated 3.4e-3 rel err vs the fp32 reference on CPU):
z state, weights, planes, k tiles.  No PSUM accumulator state, no GpSimd
ops (its software tensor_scalar measured 4.6us each in the v1 trace), no
fp32 matmuls (4 cycles/row vs 1 for fp16).

Per eval: e = W1 @ zin (PE) -> ELU -> a2 = W2 @ h1 (PE) -> ReLU ->
a3 = W3' @ h2 (PE) -> k' = (a3 + b3') * plane (DVE) -> zin_next =
3|1.5 * k' + z (DVE).  Butcher weights (dt/6, dt/3) are folded into two
scaled copies of W3, so planes are the *raw* spline derivative on the
half-step grid (s = g/8) and the zin scalars are 3 / 1.5 / 3 exactly.

ELU has no native table; two decompositions, selectable per eval to
balance the Scalar(ACT) and Vector(DVE) engines:
  form A (ACT-heavy):  r = Relu(a1 + b1); u = Relu(1 - exp(a1 + b1));
                       h1 = r - u                       (2 ACT + 1 DVE)
  form B (DVE-heavy):  e = exp(a1 + b1); t = min(e,1) - 1;
                       h1 = max(a1 + b1, t)             (1 ACT + 2 DVE)
Both are exact (exp overflow to inf is absorbed by min/relu clamps).

z update: z' = ((z + (k1'+k2')) + (k3'+k4')) with the first add issued
mid-step (off the critical chain).
"""

import os
import sys

sys.path.insert(0, "/opt/trn_rl_repo")

import numpy as np

import concourse.bass as bass
import concourse.bacc as bacc
import concourse.mybir as mybir
import concourse.tile as tile
from concourse.bass_utils import run_bass_kernel_spmd

N_CORES = 8
B, P, C, H, O = 4096, 64, 128, 128, 10
BC = B // N_CORES  # 512 samples per core
SPP = 4  # RK4 steps per spline piece
DT = 1.0 / SPP

F32 = mybir.dt.float32
F16 = mybir.dt.float16
AL = mybir.AluOpType
AF = mybir.ActivationFunctionType

NSUB = int(os.environ.get("CDE_NSUB", "4"))
# ELU form per RK4 eval.  BBAA (cheap-DVE A forms in evals 2-3) beats
# AABB by 2.7%: the z-update ops cluster in the back half of each step,
# so lighter DVE forms there let z' land earlier (measured 4.250 vs
# 4.367 ms)
FORMS = os.environ.get("CDE_FORMS", "BBAA")
RELU2 = os.environ.get("CDE_RELU2", "dddd")  # per eval: 'a' ACT | 'd' DVE ts
ZIN = os.environ.get("CDE_ZIN", "pe")  # 'dve' stt | 'pe' accumulate-matmul
# 1 = persistent per-chain e-bank with correction-form accumulation
# (e3 = e2 + 1.5*W1@k2' - 3*W1@k1'), saving the per-eval W1@z reseed
ESTATE = os.environ.get("CDE_ESTATE", "1") == "1"
# offload the off-chain t12 = k1'+k2' adds to the (idle) gpsimd engine
GP12 = os.environ.get("CDE_GP12", "0") == "1"
# 1 = emit each chain's z-tail (t34, z') immediately after its own k4
# drain instead of after all chains' drains
TAIL = os.environ.get("CDE_TAIL", "0") == "1"

# fp16 pack layout (free-dim offsets): z0 | w1 | w2 | w3_6 | w3_3 | wr
_O_Z0 = 0
_O_W1 = _O_Z0 + BC
_O_W2 = _O_W1 + H
_O_W36 = _O_W2 + H
_O_W33 = _O_W36 + C
_O_WR = _O_W33 + C
_O_W13 = _O_WR + O   # 3*W1 (zin 'pe' route)
_O_W115 = _O_W13 + H  # 1.5*W1
_O_W1N3 = _O_W115 + H  # -3*W1 (estate correction)
_O_W1N15 = _O_W1N3 + H  # -1.5*W1
P16_TOT = _O_W1N15 + H
# fp32 pack layout: b1 | b2 | b3_6 | b3_3 | br
P32_TOT = 5


def build_kernel(n_pieces: int = P, nsub: int = NSUB, forms: str = FORMS,
                 relu2: str = RELU2, zin_route: str = ZIN,
                 b1z: bool = True, b2z: bool = True,
                 b3z: bool = True, estate: bool = ESTATE) -> bass.Bass:
    fd = BC // nsub
    if not b2z:
        relu2 = "aaaa"

    nc = bacc.Bacc("TRN2")

    pack16d = nc.dram_tensor("pack16", [C, P16_TOT], F16, kind="ExternalInput")
    pack32d = nc.dram_tensor("pack32", [C, P32_TOT], F32, kind="ExternalInput")
    # host-precomputed spline-derivative planes: grid g at s=g/8 per piece,
    # plus the s=1 plane of the final piece (pl1)
    pld = nc.dram_tensor("pl", [n_pieces, C, 8, BC], F16, kind="ExternalInput")
    pl1d = nc.dram_tensor("pl1", [C, BC], F16, kind="ExternalInput")
    outf = nc.dram_tensor("outf", [O, BC], F32, kind="ExternalOutput")

    with tile.TileContext(nc) as tc:
        with tc.tile_pool(name="const", bufs=1) as const:
            pk16 = const.tile([C, P16_TOT], F16)
            pk32 = const.tile([C, P32_TOT], F32)
            nc.sync.dma_start(pk16[:], pack16d[:])
            nc.sync.dma_start(pk32[:], pack32d[:])

            z0_sl = pk16[:, _O_Z0:_O_Z0 + BC]
            w1 = pk16[:, _O_W1:_O_W1 + H]
            w2 = pk16[:, _O_W2:_O_W2 + H]
            w36 = pk16[:, _O_W36:_O_W36 + C]
            w33 = pk16[:, _O_W33:_O_W33 + C]
            wr = pk16[:, _O_WR:_O_WR + O]
            w13 = pk16[:, _O_W13:_O_W13 + H]
            w115 = pk16[:, _O_W115:_O_W115 + H]
            w1n3 = pk16[:, _O_W1N3:_O_W1N3 + H]
            w1n15 = pk16[:, _O_W1N15:_O_W1N15 + H]
            b1 = pk32[:, 0:1]
            b2 = pk32[:, 1:2]
            b36 = pk32[:, 2:3]
            b33 = pk32[:, 3:4]
            br = pk32[0:O, 4:5]

            _kernel_body(nc, tc, n_pieces, nsub, fd, forms, relu2, zin_route,
                         b1z, b3z, estate, z0_sl, pld, pl1d, outf,
                         w1, w2, w36, w33, wr, w13, w115, w1n3, w1n15,
                         b1, b2, b36, b33, br)
    nc.finalize()
    return nc


def _kernel_body(nc, tc, n_pieces, nsub, fd, forms, relu2, zin_route,
                 b1z, b3z, estate, z0_sl, pld, pl1d, outf,
                 w1, w2, w36, w33, wr, w13, w115, w1n3, w1n15,
                 b1, b2, b36, b33, br):
    import contextlib
    ctx = contextlib.ExitStack()
    with ctx:
        planep = ctx.enter_context(tc.tile_pool(name="plane", bufs=3))
        zp = ctx.enter_context(tc.tile_pool(name="zsb", bufs=2))
        hp = ctx.enter_context(tc.tile_pool(name="hwork", bufs=2))
        kp = ctx.enter_context(tc.tile_pool(name="kwork", bufs=2))
        outp = ctx.enter_context(tc.tile_pool(name="outw", bufs=1))
        # one PSUM bank per chain, reused across the MLP stages of an
        # eval (each stage's matmul overwrites it only after the previous
        # stage's reader is done -- which the serial chain guarantees).
        # The readout reuses chain 0's bank at the very end.
        pa = ctx.enter_context(tc.tile_pool(name="pa", bufs=1, space="PSUM"))
        if estate:
            # persistent per-chain e-bank: holds W1@zin for the current
            # eval, updated by correction-form accumulating matmuls
            ebp = ctx.enter_context(tc.tile_pool(name="ebp", bufs=1,
                                                 space="PSUM"))

        plane_tiles = {}

        def load_piece(p):
            pl = planep.tile([C, 8 * BC], F16, name=f"plane_{p}", tag="plane")
            nc.gpsimd.dma_start(pl[:], pld[p])
            plane_tiles[p] = pl

        def grid_ap(p, g):
            return plane_tiles[p][:, g * BC:(g + 1) * BC]

        # prologue: first two pieces' planes in flight + the final s=1 plane
        load_piece(0)
        if n_pieces > 1:
            load_piece(1)
        extra_s1 = planep.tile([C, BC], F16, name="plane_s1", tag="plane_s1")
        nc.gpsimd.dma_start(extra_s1[:], pl1d[:])

        # current z per chain, [C, fd] fp16 SBUF
        z_sb = [z0_sl[:, c * fd:(c + 1) * fd] for c in range(nsub)]

        def psl(ap, c):  # slice a full-BC plane/coef column range for chain c
            return ap[:, c * fd:(c + 1) * fd]

        # ================= main time loop =================
        for p in range(n_pieces):
            for j in range(SPP):
                step = p * SPP + j

                # plane prefetch (gpsimd DMA queue, idle engine)
                if j == 0 and p + 2 < n_pieces:
                    load_piece(p + 2)

                # plane refs for this step's 4 evals
                g1 = grid_ap(p, 2 * j)
                g23 = grid_ap(p, 2 * j + 1)
                if j < SPP - 1:
                    g4 = grid_ap(p, 2 * j + 2)
                elif p + 1 < n_pieces:
                    g4 = grid_ap(p + 1, 0)
                else:
                    g4 = extra_s1[:]

                z_new = [zp.tile([C, fd], F16, name=f"z_{step}_{c}",
                                 tag=f"z{c}") for c in range(nsub)]
                kt = [[kp.tile([C, fd], F16, name=f"k{i}_{step}_{c}",
                               tag=f"k{i}_{c}") for c in range(nsub)]
                      for i in range(4)]
                t12 = [kp.tile([C, fd], F16, name=f"t12_{step}_{c}",
                               tag=f"t12_{c}") for c in range(nsub)]
                t34 = [kp.tile([C, fd], F16, name=f"t34_{step}_{c}",
                               tag=f"t34_{c}") for c in range(nsub)]
                za = [kp.tile([C, fd], F16, name=f"za_{step}_{c}",
                              tag=f"za_{c}") for c in range(nsub)]

                zin = z_sb
                if estate:
                    ebt = [ebp.tile([H, fd], F32, name=f"eb_{step}_{c}",
                                    tag=f"eb{c}") for c in range(nsub)]
                for ev in range(4):
                    w3x = w36 if ev in (0, 3) else w33
                    b3x = b36 if ev in (0, 3) else b33
                    gpl = (g1, g23, g23, g4)[ev]
                    form = forms[ev]

                    h1 = [hp.tile([C, fd], F16, name=f"h1_{step}_{ev}_{c}",
                                  tag=f"h1_{c}") for c in range(nsub)]

                    if estate:
                        # correction-form updates of the persistent e-bank:
                        # ev0: W1@z; ev1: +3W1@k1'; ev2: +1.5W1@k2'-3W1@k1';
                        # ev3: +3W1@k3'-1.5W1@k2'.  The +/- pairs cancel
                        # exactly in fp32 PSUM (same inputs, negated weight).
                        corr = ([(w1, None)], [(w13, 0)],
                                [(w115, 1), (w1n3, 0)],
                                [(w13, 2), (w1n15, 1)])[ev]
                        for wi, (wx, ki) in enumerate(corr):
                            last = (ev == 3 and wi == len(corr) - 1)
                            for c in range(nsub):
                                rhs = (z_sb[c][:] if ki is None
                                       else kt[ki][c][:])
                                nc.tensor.matmul(ebt[c][:], wx, rhs,
                                                 start=(ev == 0), stop=last,
                                                 skip_group_check=True)
                        a1 = ebt
                    elif zin_route == "pe" and ev > 0:
                        # e = W1 @ z + sc*W1 @ k_prev  (two accumulating
                        # matmuls; zin never materialized).  Same-weight
                        # matmuls grouped so LDWEIGHTS doesn't alternate.
                        a1 = [pa.tile([H, fd], F32,
                                      name=f"a1_{step}_{ev}_{c}",
                                      tag=f"a{c}") for c in range(nsub)]
                        w1x = w115 if ev == 2 else w13
                        for c in range(nsub):
                            nc.tensor.matmul(a1[c][:], w1, z_sb[c][:],
                                             start=True, stop=False)
                        for c in range(nsub):
                            nc.tensor.matmul(a1[c][:], w1x, kt[ev - 1][c][:],
                                             start=False, stop=True)
                    else:
                        a1 = [pa.tile([H, fd], F32,
                                      name=f"a1_{step}_{ev}_{c}",
                                      tag=f"a{c}") for c in range(nsub)]
                        for c in range(nsub):
                            nc.tensor.matmul(a1[c][:], w1, zin[c][:],
                                             start=True, stop=True)
                    # ELU
                    if form == "A":
                        for c in range(nsub):
                            eb = hp.tile([C, fd], F16, name=f"e_{step}_{ev}_{c}",
                                         tag=f"e_{c}")
                            rb = hp.tile([C, fd], F16, name=f"r_{step}_{ev}_{c}",
                                         tag=f"r_{c}")
                            nc.scalar.activation(eb[:], a1[c][:], AF.Exp,
                                                 bias=b1, scale=1.0)
                            nc.scalar.activation(rb[:], a1[c][:], AF.Relu,
                                                 bias=b1, scale=1.0)
                            ub = hp.tile([C, fd], F16, name=f"u_{step}_{ev}_{c}",
                                         tag=f"u_{c}")
                            nc.scalar.activation(ub[:], eb[:], AF.Relu,
                                                 bias=1.0, scale=-1.0)
                            nc.vector.tensor_tensor(h1[c][:], rb[:], ub[:],
                                                    AL.subtract)
                    else:  # form B
                        for c in range(nsub):
                            eb = hp.tile([C, fd], F16, name=f"e_{step}_{ev}_{c}",
                                         tag=f"e_{c}")
                            tb = hp.tile([C, fd], F16, name=f"t_{step}_{ev}_{c}",
                                         tag=f"t_{c}")
                            nc.scalar.activation(eb[:], a1[c][:], AF.Exp,
                                                 bias=b1, scale=1.0)
                            nc.vector.tensor_scalar(tb[:], eb[:],
                                                    1.0, -1.0, AL.min, AL.add)
                            if b1z:
                                nc.vector.tensor_tensor(
                                    h1[c][:], a1[c][:], tb[:], AL.max)
                            else:
                                nc.vector.scalar_tensor_tensor(
                                    h1[c][:], a1[c][:], b1, tb[:],
                                    AL.add, AL.max)

                    # L2 matmul + ReLU (a-bank reused: write waits h1 read)
                    a2 = [pa.tile([H, fd], F32, name=f"a2_{step}_{ev}_{c}",
                                  tag=f"a{c}") for c in range(nsub)]
                    h2 = [hp.tile([C, fd], F16, name=f"h2_{step}_{ev}_{c}",
                                  tag=f"h2_{c}") for c in range(nsub)]
                    for c in range(nsub):
                        nc.tensor.matmul(a2[c][:], w2, h1[c][:],
                                         start=True, stop=True)
                        if relu2[ev] == "a":
                            nc.scalar.activation(h2[c][:], a2[c][:], AF.Relu,
                                                 bias=b2, scale=1.0)
                        else:
                            nc.vector.tensor_scalar(h2[c][:], a2[c][:],
                                                    0.0, None, AL.max)

                    # L3 matmul + k-drain (+ zin for next eval)
                    a3 = [pa.tile([C, fd], F32, name=f"a3_{step}_{ev}_{c}",
                                  tag=f"a{c}") for c in range(nsub)]
                    want_zin = (ev < 3 and zin_route != "pe" and not estate)
                    zin_next = ([kp.tile([C, fd], F16,
                                         name=f"zin_{step}_{ev}_{c}",
                                         tag=f"zin{ev}_{c}")
                                 for c in range(nsub)] if want_zin else None)
                    zin_scale = (3.0, 1.5, 3.0, 0.0)[ev]
                    for c in range(nsub):
                        nc.tensor.matmul(a3[c][:], w3x, h2[c][:],
                                         start=True, stop=True)
                    for c in range(nsub):
                        if b3z:
                            nc.vector.tensor_tensor(
                                kt[ev][c][:], a3[c][:], psl(gpl, c), AL.mult)
                        else:
                            nc.vector.scalar_tensor_tensor(
                                kt[ev][c][:], a3[c][:], b3x, psl(gpl, c),
                                AL.add, AL.mult)
                        if want_zin:
                            nc.vector.scalar_tensor_tensor(
                                zin_next[c][:], kt[ev][c][:], zin_scale,
                                z_sb[c][:], AL.mult, AL.add)
                        if ev == 3 and TAIL:
                            # fuse the z tail per chain right behind its
                            # k4 drain: z'_c dequeues before chains c+1..
                            # even drain, releasing each chain into the
                            # next step ~3 ops earlier
                            nc.vector.tensor_tensor(t34[c][:], kt[2][c][:],
                                                    kt[3][c][:], AL.add)
                            nc.vector.tensor_tensor(z_new[c][:], za[c][:],
                                                    t34[c][:], AL.add)
                    # mid-step z partials (off critical chain)
                    if ev == 1:
                        eng12 = nc.gpsimd if GP12 else nc.vector
                        for c in range(nsub):
                            eng12.tensor_tensor(t12[c][:], kt[0][c][:],
                                                kt[1][c][:], AL.add)
                    if ev == 2:
                        for c in range(nsub):
                            nc.vector.tensor_tensor(za[c][:], z_sb[c][:],
                                                    t12[c][:], AL.add)
                    if ev == 3 and not TAIL:
                        for c in range(nsub):
                            nc.vector.tensor_tensor(t34[c][:], kt[2][c][:],
                                                    kt[3][c][:], AL.add)
                            nc.vector.tensor_tensor(z_new[c][:], za[c][:],
                                                    t34[c][:], AL.add)
                    if want_zin:
                        zin = zin_next
                z_sb = z_new

        op = pa.tile([O, BC], F32, name="ops", tag="a0")
        for c in range(nsub):
            nc.tensor.matmul(op[:, c * fd:(c + 1) * fd], wr, z_sb[c][:],
                             start=True, stop=True)
        out_sb = outp.tile([O, BC], F32, name="out_sb")
        nc.scalar.activation(out_sb[:], op[:], AF.Identity, bias=br, scale=1.0)
        nc.sync.dma_start(outf[:], out_sb[:])


# ---------------------------------------------------------------------------
# host side
# ---------------------------------------------------------------------------

_BUILT = {}


def _get_kernel(n_pieces=P, nsub=NSUB, forms=FORMS, relu2=RELU2, zin=ZIN,
                b1z=True, b2z=True, b3z=True, estate=ESTATE):
    key = (n_pieces, nsub, forms, relu2, zin, b1z, b2z, b3z, estate)
    if key not in _BUILT:
        _BUILT[key] = build_kernel(n_pieces, nsub, forms, relu2, zin,
                                   b1z, b2z, b3z, estate)
    return _BUILT[key]


def _prep_inputs(z0, coeffs, W1, b1, W2, b2, W3, b3, Wr, br, n_pieces=P):
    z0 = np.asarray(z0, np.float32)
    coeffs = np.asarray(coeffs, np.float32)

    z0c = z0.reshape(N_CORES, BC, C).transpose(0, 2, 1)  # [core, C, BC]

    # spline-derivative planes dX(s) = c1 + 2s c2 + 3s^2 c3 on s = g/8,
    # fp16, laid out [core, P, C, 8, BC]
    cc = coeffs[:, :n_pieces, :, 1:4].reshape(N_CORES, BC, n_pieces, C, 3)
    cc = cc.transpose(0, 2, 3, 4, 1)  # [core, P, C, 3, BC]
    s = np.arange(8, dtype=np.float32) / 8.0
    planes = (cc[:, :, :, None, 0, :]
              + (2.0 * s)[None, None, None, :, None] * cc[:, :, :, None, 1, :]
              + (3.0 * s * s)[None, None, None, :, None]
              * cc[:, :, :, None, 2, :]).astype(np.float16)
    pl1 = (cc[:, n_pieces - 1, :, 0] + 2.0 * cc[:, n_pieces - 1, :, 1]
           + 3.0 * cc[:, n_pieces - 1, :, 2]).astype(np.float16)  # s=1

    pack16 = np.zeros((N_CORES, C, P16_TOT), np.float16)
    pack16[:, :, _O_Z0:_O_Z0 + BC] = z0c.astype(np.float16)
    pack16[:, :, _O_W1:_O_W1 + H] = np.asarray(W1, np.float16)
    pack16[:, :, _O_W2:_O_W2 + H] = np.asarray(W2, np.float16)
    pack16[:, :, _O_W36:_O_W36 + C] = (np.asarray(W3, np.float32)
                                       * (DT / 6.0)).astype(np.float16)
    pack16[:, :, _O_W33:_O_W33 + C] = (np.asarray(W3, np.float32)
                                       * (DT / 3.0)).astype(np.float16)
    pack16[:, :H, _O_WR:_O_WR + O] = np.asarray(Wr, np.float16)
    w13h = (3.0 * np.asarray(W1, np.float32)).astype(np.float16)
    w115h = (1.5 * np.asarray(W1, np.float32)).astype(np.float16)
    pack16[:, :, _O_W13:_O_W13 + H] = w13h
    pack16[:, :, _O_W115:_O_W115 + H] = w115h
    pack16[:, :, _O_W1N3:_O_W1N3 + H] = -w13h
    pack16[:, :, _O_W1N15:_O_W1N15 + H] = -w115h

    pack32 = np.zeros((C, P32_TOT), np.float32)
    pack32[:H, 0] = np.asarray(b1, np.float32)
    pack32[:H, 1] = np.asarray(b2, np.float32)
    pack32[:C, 2] = np.asarray(b3, np.float32) * (DT / 6.0)
    pack32[:C, 3] = np.asarray(b3, np.float32) * (DT / 3.0)
    pack32[:O, 4] = np.asarray(br, np.float32)

    in_maps = []
    for c in range(N_CORES):
        in_maps.append({
            "pack16": np.ascontiguousarray(pack16[c]),
            "pack32": pack32,
            "pl": np.ascontiguousarray(planes[c]),
            "pl1": np.ascontiguousarray(pl1[c]),
        })
    return in_maps


def run(z0, coeffs, W1, b1, W2, b2, W3, b3, Wr, br,
        n_pieces=P, nsub=NSUB, forms=FORMS, relu2=RELU2, zin=ZIN,
        estate=ESTATE, trace=False):
    b1z = bool(np.all(np.asarray(b1) == 0.0))
    b2z = bool(np.all(np.asarray(b2) == 0.0))
    b3z = bool(np.all(np.asarray(b3) == 0.0))
    nc = _get_kernel(n_pieces, nsub, forms, relu2, zin, b1z, b2z, b3z, estate)
    in_maps = _prep_inputs(z0, coeffs, W1, b1, W2, b2, W3, b3, Wr, br,
                           n_pieces=n_pieces)
    res = run_bass_kernel_spmd(nc, in_maps, core_ids=list(range(N_CORES)),
                               trace=trace)
    outs = [res.results[c]["outf"] for c in range(N_CORES)]  # [O, BC]
    out = np.concatenate([o.T for o in outs], axis=0)  # [B, O]
    return np.asarray(out, np.float32), res


def kernel(z0, coeffs, W1, b1, W2, b2, W3, b3, Wr, br):
    out, _ = run(z0, coeffs, W1, b1, W2, b2, W3, b3, Wr, br)
    return out
